# revision 34
# baseline (speedup 1.0000x reference)
"""CRvNN balanced-tree reduction on 8 TRN2 NeuronCores (pure data parallel).

Reference computation (per batch element, S=512 leaves, D=1024, CH=2048):
  x = LN((seq * mask) @ W_init + b_init) * mask
  9 levels of pairwise tree reduction; per level, for each adjacent pair
  (l, r):  cat=[l,r] -> h=relu(cat@W1+b1) -> contents=h@W2+b2 ->
  (f1,f2,i,parent) -> LN(f1*l + f2*r + i*parent) blended by the pair mask.

Distribution: batch 64 -> 8 elements per core, weights replicated; the tree
reduction is independent per batch element, so there are no collectives.

Device kernel (per core, all 9 levels + init fused in one NEFF):
  - fp16 matmuls (PE runs fp16 at 4x the fp32 rate), f32 PSUM accumulate,
    all gating/blending/LayerNorm math in f32 on ACT/DVE.
  - x lives in DRAM between levels as f32 (blend inputs) plus an fp16 mirror
    that feeds DMA-transposed catT tiles for the next level's matmuls.
  - stage 1: hT[CH, rows] = W1^T @ catT, W1 streamed per 128-col slice
    (stationary), catT moving.  relu+b1 fused in the PSUM->SBUF evict.
  - stage 2: contents[rows, 4D] = h @ W2, hT slices stationary, W2
    SBUF-resident (fp16, 16 MiB), evicted through sigmoid into the gates.
"""

import os
import sys

for _p in ("/opt/trn_rl_repo", "/root/.axon_site/_ro/trn_rl_repo"):
    if os.path.isdir(_p) and _p not in sys.path:
        sys.path.append(_p)

import numpy as np

import concourse.bass as bass
import concourse.bacc as bacc_mod
import concourse.mybir as mybir
import concourse.tile as tile
from concourse.bass_utils import run_bass_kernel_spmd
from concourse.masks import make_identity
from concourse.tile_autobufs import add_dep_helper

# ---------------------------------------------------------------------------
# Patch: this walrus build rejects >2 sem waits on one instruction; the Tile
# exit drain carries one wait per active proc.  Split the excess onto SP nops.
from concourse.tile import ScopedClock


def _patched_drain_and_barrier(self, tick_clock, wait_clock):
    nc = self.nc
    drain_inst = nc.sync.drain()
    wait_clock.add_sem_waits(
        drain_inst.ins, ScopedClock({None: tick_clock.global_clock})
    )
    si = drain_inst.ins.sync_info
    waits = si.on_wait if si is not None else None
    extra = []
    while waits is not None and len(waits) > 1:
        extra.append(waits.pop())
    for sw in extra:
        nop = nc.sync.nop(nofuse=True, hint="drain_wait_split")
        nsi = nop.ins.sync_info
        if nsi is None or nsi.on_wait is None:
            nop.ins.sync_info = mybir.SyncInfo(on_wait=[sw], on_update=[])
        else:
            nsi.on_wait.append(sw)

    nc.all_engine_barrier()
    assert self.sems is not None
    popped = nc._tile_sem_poison_stack.pop()
    assert popped is self._sem_poison
    nc.clear_and_free_semaphores(list(self.sems.allocated().values()))
    nc.all_engine_barrier()


tile.TileContext._drain_and_barrier = _patched_drain_and_barrier
# ---------------------------------------------------------------------------

F32 = mybir.dt.float32
F16 = mybir.dt.float16
AF = mybir.ActivationFunctionType
ALU = mybir.AluOpType

D = 1024
CH = 2048
LN_EPS = 1e-5
P = 128
KT1 = (2 * D) // P   # 16 k-tiles for stage 1 / cat
MT1 = CH // P        # 16 m-tiles for stage 1 output (hT partitions)
KT2 = CH // P        # 16 k-tiles for stage 2
KT0 = D // P         # 8 k-tiles for the init matmul


def build_graph(nl, S, chunk=512, use_mask=False, use_gb=False,
                use_b2=False, use_binit=False):
    """Build the per-core Bass graph.  nl = local batch, S = sequence length.

    Flag args compile in the generic paths (pair-mask blending, ln scale/bias,
    matmul biases); they are off for the graded inputs (mask=1, g=1, b=0).
    """
    nlevels = S.bit_length() - 1
    assert S == 1 << nlevels
    R0 = nl * S

    nc = bacc_mod.Bacc()
    IC0 = min(512, R0)
    xt0_ext = nc.declare_dram_parameter("xt0", [P, R0 // IC0, KT0, IC0], F16, isOutput=False)
    winit_ext = nc.declare_dram_parameter("winit", [P, KT0, D], F16, isOutput=False)
    w1_ext = nc.declare_dram_parameter("w1", [MT1, P, KT1, P], F16, isOutput=False)
    w2_ext = nc.declare_dram_parameter("w2", [P, KT2, 4 * D], F16, isOutput=False)
    b1c_ext = nc.declare_dram_parameter("b1c", [P, MT1], F32, isOutput=False)
    if use_gb:
        lng_ext = nc.declare_dram_parameter("lng", [1, D], F32, isOutput=False)
        lnb_ext = nc.declare_dram_parameter("lnb", [1, D], F32, isOutput=False)
    if use_b2:
        b2r_ext = nc.declare_dram_parameter("b2r", [1, 4 * D], F16, isOutput=False)
    if use_binit:
        bir_ext = nc.declare_dram_parameter("bir", [1, D], F16, isOutput=False)
    if use_mask:
        # maskc: per-row mask after init ([P, R0/P] column-tiled)
        # mrc: per-level pair masks, each level padded to whole columns
        maskc_ext = nc.declare_dram_parameter("maskc", [P, max(1, R0 // P)], F32, isOutput=False)
        n_mr_cols = sum(max(1, (nl * (S >> (lv + 1))) // P) for lv in range(nlevels))
        mrc_ext = nc.declare_dram_parameter("mrc", [P, n_mr_cols], F32, isOutput=False)
    out_ext = nc.declare_dram_parameter("out", [nl, D], F32, isOutput=True)

    # DRAM scratch: ping/pong x (f16; feeds both the transposed matmul
    # loads and the l/r blend reads)
    xh = [nc.dram_tensor(f"xh{i}", [R0, D], F16) for i in range(2)]

    with tile.TileContext(nc) as tc:
        import contextlib
        with contextlib.ExitStack() as ctx:
            const = ctx.enter_context(tc.tile_pool(name="const", bufs=1))
            w2p = ctx.enter_context(tc.tile_pool(name="w2p", bufs=1))
            w1p = ctx.enter_context(tc.tile_pool(name="w1p", bufs=2))
            catp = ctx.enter_context(tc.tile_pool(name="catp", bufs=1))
            htp = ctx.enter_context(tc.tile_pool(name="htp", bufs=1))
            lrp = ctx.enter_context(tc.tile_pool(name="lrp", bufs=1))
            gp = ctx.enter_context(tc.tile_pool(name="gp", bufs=1))
            op_bufs = 1 if (use_gb or use_b2 or use_binit) else 2
            op = ctx.enter_context(tc.tile_pool(name="op", bufs=op_bufs))
            sp = ctx.enter_context(tc.tile_pool(name="sp", bufs=4))
            ps1 = ctx.enter_context(tc.tile_pool(name="ps1", bufs=2, space="PSUM"))
            ps2 = ctx.enter_context(tc.tile_pool(name="ps2", bufs=3, space="PSUM"))

            # --- resident constants -------------------------------------
            # (the 16 MiB w2 load is EMITTED after the init loop so it does
            # not head-of-line-block the init's input DMAs; it is only
            # needed from level 1 stage 2 onward)
            w2_sb = w2p.tile([P, KT2, 4 * D], F16)
            # winit borrows the (init-time idle) hT slot: same 16 KiB/part
            winit_sb = htp.tile([P, KT0, D], F16, tag="hT")
            nc.sync.dma_start(winit_sb[:], winit_ext[:])
            b1c_sb = const.tile([P, MT1], F32)
            eps_sb = const.tile([P, 1], F32)
            nc.vector.memset(eps_sb[:], LN_EPS)
            ident_sb = const.tile([P, P], F16)
            make_identity(nc, ident_sb)
            if use_gb:
                g_sb = const.tile([P, D], F32)
                b_sb = const.tile([P, D], F32)
                nc.sync.dma_start(g_sb[:], lng_ext[:].to_broadcast((P, D)))
                nc.sync.dma_start(b_sb[:], lnb_ext[:].to_broadcast((P, D)))
            if use_b2:
                b2_sb = const.tile([1, 4 * D], F16)
                nc.sync.dma_start(b2_sb[:], b2r_ext[:])
            if use_binit:
                bir_sb = const.tile([1, D], F16)
                nc.sync.dma_start(bir_sb[:], bir_ext[:])
            if use_b2 or use_binit:
                ones_sb = const.tile([1, P], F16)
                nc.vector.memset(ones_sb[:], 1.0)
            if use_mask:
                maskc_sb = const.tile([P, max(1, R0 // P)], F32)
                nc.sync.dma_start(maskc_sb[:], maskc_ext[:])
                mrc_sb = const.tile([P, n_mr_cols], F32)
                nc.sync.dma_start(mrc_sb[:], mrc_ext[:])

            def layer_norm_store(x_in, rw, rt0, dst_idx, last, mask_col=None,
                                 l_tile=None, mr_col=None, scratch=None):
                """LN(x_in[:rw]) (+g/b), optional mask blend with l_tile,
                then store f32 + f16 mirrors (or `out` on the last level).
                x_in may be SBUF or PSUM; scratch: SBUF f32 tile for temps.
                Returns the f16 output tile (None on the last level)."""
                stats = sp.tile([P, 2, 6], F32)
                mv = sp.tile([P, 2], F32)
                nc.vector.bn_stats(out=stats[:rw, 0, :], in_=x_in[:rw, 0:512])
                nc.vector.bn_stats(out=stats[:rw, 1, :], in_=x_in[:rw, 512:1024])
                nc.vector.bn_aggr(out=mv[:rw], in_=stats[:rw])
                rs = sp.tile([P, 1], F32)
                nc.scalar.activation(rs[:rw], mv[:rw, 1:2], AF.Sqrt, bias=eps_sb[:rw])
                nc.vector.reciprocal(rs[:rw], rs[:rw])
                direct16 = not (last or use_gb or mask_col is not None
                                or mr_col is not None)
                if direct16:
                    xo16 = op.tile([P, D], F16)
                    nc.vector.tensor_scalar(
                        out=xo16[:rw], in0=x_in[:rw],
                        scalar1=mv[:rw, 0:1], scalar2=rs[:rw],
                        op0=ALU.subtract, op1=ALU.mult,
                    )
                    nc.sync.dma_start(xh[dst_idx][rt0:rt0 + rw, :], xo16[:rw])
                    return xo16
                xo = op.tile([P, D], F32)
                nc.vector.tensor_scalar(
                    out=xo[:rw], in0=x_in[:rw],
                    scalar1=mv[:rw, 0:1], scalar2=rs[:rw],
                    op0=ALU.subtract, op1=ALU.mult,
                )
                if use_gb:
                    nc.vector.tensor_mul(xo[:rw], xo[:rw], g_sb[:rw])
                    nc.vector.tensor_add(xo[:rw], xo[:rw], b_sb[:rw])
                if mask_col is not None:
                    # init re-mask: x *= mask
                    nc.vector.tensor_scalar_mul(xo[:rw], xo[:rw], mask_col)
                if mr_col is not None:
                    # x = l + mr*(x - l)
                    t = scratch
                    nc.vector.tensor_sub(t[:rw], xo[:rw], l_tile[:rw])
                    nc.vector.tensor_scalar_mul(t[:rw], t[:rw], mr_col)
                    nc.vector.tensor_add(xo[:rw], l_tile[:rw], t[:rw])
                if last:
                    nc.sync.dma_start(out_ext[rt0:rt0 + rw, :], xo[:rw])
                    return None
                xo16 = op.tile([P, D], F16)
                nc.vector.tensor_copy(xo16[:rw], xo[:rw])
                nc.sync.dma_start(xh[dst_idx][rt0:rt0 + rw, :], xo16[:rw])
                return xo16

            # --- init: y = LN((seq*mask) @ W_init + b_init) * mask ------
            # one persistent tile, ping-pong between column halves so the
            # next chunk's DMA overlaps this chunk's matmuls
            IC = IC0
            xt0_sb = catp.tile([P, KT0, 2, IC], F16, tag="catT")
            n_init_chunks = (R0 + IC - 1) // IC
            xt0_dmas = []
            for ci, c0 in enumerate(range(0, R0, IC)):
                cw = min(IC, R0 - c0)
                half = ci % 2
                xt0_c = xt0_sb[:, :, half, :]
                xt0_dmas.append(nc.sync.dma_start(
                    xt0_c[:, :, :cw], xt0_ext[:, ci, :, :cw]
                ))

                for rt in range((cw + P - 1) // P):
                    rw = min(P, cw - rt * P)
                    r0 = rt * P
                    psy = ps2.tile([P, D], F32, tag="psq")
                    for k in range(KT0):
                        st = (k == 0)
                        sp_ = (k == KT0 - 1) and not use_binit
                        nc.tensor.matmul(
                            psy[:rw, 0:512],
                            xt0_c[:, k, r0:r0 + rw],
                            winit_sb[:, k, 0:512], start=st, stop=sp_)
                        nc.tensor.matmul(
                            psy[:rw, 512:1024],
                            xt0_c[:, k, r0:r0 + rw],
                            winit_sb[:, k, 512:1024], start=st, stop=sp_)
                    if use_binit:
                        nc.tensor.matmul(psy[:rw, 0:512], ones_sb[:1, :rw],
                                         bir_sb[:1, 0:512], start=False, stop=True)
                        nc.tensor.matmul(psy[:rw, 512:1024], ones_sb[:1, :rw],
                                         bir_sb[:1, 512:1024], start=False, stop=True)
                    # quick ACT evict: frees PSUM sooner, LN runs on SBUF
                    yw = gp.tile([P, D], F32, tag="gate0")
                    nc.scalar.activation(yw[:rw], psy[:rw], AF.Copy)
                    gcol = (c0 + r0) // P
                    layer_norm_store(
                        yw, rw, c0 + r0, 0, last=False,
                        mask_col=maskc_sb[:rw, gcol:gcol + 1] if use_mask else None,
                    )

            # deferred resident loads (needed from level 1 onward); the
            # 16 MiB w2 load rides the SWDGE queues so it cannot starve
            # the init input stream on the HWDGE queues
            w2dma = nc.gpsimd.dma_start(w2_sb[:], w2_ext[:])
            # hold the 16 MiB w2 load back until the init stream is rolling,
            # so it does not monopolize HBM before the first matmul
            add_dep_helper(w2dma.ins, xt0_dmas[min(1, len(xt0_dmas) - 1)].ins,
                           reason="w2 residency load after init stream start")
            nc.sync.dma_start(b1c_sb[:], b1c_ext[:])

            # --- tree levels -------------------------------------------
            src, dst = 0, 1
            mr_off = 0
            prev16 = None   # f16 output tiles of the previous level (tail)
            for lv in range(nlevels):
                rows = nl * (S >> (lv + 1))   # parent count this level
                rows_x = 2 * rows             # child count
                last = (lv == nlevels - 1)
                # tail levels: children live in <=2 SBUF tiles; transpose
                # on the PE instead of round-tripping through DRAM
                onchip = (rows_x <= 2 * P and prev16 is not None
                          and len(prev16) * P >= rows_x)
                cat_view = xh[src][:].rearrange("(r two) d -> r (two d)", two=2)
                out16 = []
                xT = None
                if onchip:
                    xT = catp.tile([P, KT0, 2 * P], F16, tag="catT")
                    for t, (t16, cwt) in enumerate(prev16):
                        for kk in range(KT0):
                            tp = ps1.tile([P, P], F16, tag="psh")
                            nc.tensor.transpose(
                                tp[:P, :cwt], t16[:cwt, kk * P:(kk + 1) * P],
                                ident_sb[:cwt, :cwt])
                            nc.vector.tensor_copy(
                                xT[:, kk, t * P:t * P + cwt], tp[:, :cwt])
                for c0 in range(0, rows, chunk):
                    cw = min(chunk, rows - c0)
                    if not onchip:
                        # catT tiles via DMA transpose from the f16 mirror;
                        # split by row halves so the first half's transposes
                        # start before the previous level's last rows land
                        catT = catp.tile([P, KT1, chunk], F16, tag="catT")
                        cw2 = cw // 2 if cw >= 32 else cw
                        for k in range(KT1):
                            for h0 in range(0, cw, cw2):
                                hw = min(cw2, cw - h0)
                                nc.sync.dma_start_transpose(
                                    catT[:, k, h0:h0 + hw],
                                    cat_view[c0 + h0:c0 + h0 + hw,
                                             k * P:(k + 1) * P],
                                )
                    # stage 1: hT = relu(W1^T @ catT + b1)
                    hT = htp.tile([P, MT1, chunk], F16, tag="hT")
                    for m in range(MT1):
                        w1m = w1p.tile([P, KT1, P], F16)
                        nc.sync.dma_start(w1m[:], w1_ext[m, :, :, :])
                        psh = ps1.tile([P, chunk], F32, tag="psh")
                        for k in range(KT1):
                            if onchip:
                                ph = 0 if k < KT0 else 1
                                rhs = xT[:, k % KT0, ph:ph + 2 * cw - 1:2]
                            else:
                                rhs = catT[:, k, :cw]
                            nc.tensor.matmul(
                                psh[:, :cw], w1m[:, k, :], rhs,
                                start=(k == 0), stop=(k == KT1 - 1))
                        nc.scalar.activation(
                            hT[:, m, :cw], psh[:, :cw], AF.Relu,
                            bias=b1c_sb[:, m:m + 1])
                    # stage 2 + post, per 128-row tile
                    for rt in range((cw + P - 1) // P):
                        rw = min(P, cw - rt * P)
                        r0 = rt * P
                        gates = []   # f1, f2, i in SBUF; parent stays in PSUM
                        psq = None
                        for q in range(4):
                            psq = ps2.tile([P, D], F32, tag="psq")
                            for k in range(KT2):
                                st = (k == 0)
                                sp_ = (k == KT2 - 1) and not use_b2
                                nc.tensor.matmul(
                                    psq[:rw, 0:512],
                                    hT[:, k, r0:r0 + rw],
                                    w2_sb[:, k, q * D:q * D + 512],
                                    start=st, stop=sp_)
                                nc.tensor.matmul(
                                    psq[:rw, 512:1024],
                                    hT[:, k, r0:r0 + rw],
                                    w2_sb[:, k, q * D + 512:(q + 1) * D],
                                    start=st, stop=sp_)
                            if use_b2:
                                nc.tensor.matmul(
                                    psq[:rw, 0:512], ones_sb[:1, :rw],
                                    b2_sb[:1, q * D:q * D + 512],
                                    start=False, stop=True)
                                nc.tensor.matmul(
                                    psq[:rw, 512:1024], ones_sb[:1, :rw],
                                    b2_sb[:1, q * D + 512:(q + 1) * D],
                                    start=False, stop=True)
                            if q < 3:
                                gt = gp.tile([P, D], F32, tag=f"gate{q}")
                                nc.scalar.activation(gt[:rw], psq[:rw], AF.Sigmoid)
                                gates.append(gt)
                        # parent candidate = psq (q==3, raw)
                        lr = lrp.tile([P, 2, D], F16)
                        nc.sync.dma_start(
                            lr[:rw, 0, :],
                            xh[src][2 * (c0 + r0):2 * (c0 + r0 + rw):2, :])
                        nc.sync.dma_start(
                            lr[:rw, 1, :],
                            xh[src][2 * (c0 + r0) + 1:2 * (c0 + r0 + rw):2, :])
                        f1, f2, gi = gates
                        nc.vector.tensor_mul(f1[:rw], f1[:rw], lr[:rw, 0, :])
                        nc.vector.tensor_mul(f2[:rw], f2[:rw], lr[:rw, 1, :])
                        nc.vector.tensor_add(f1[:rw], f1[:rw], f2[:rw])
                        nc.vector.tensor_mul(gi[:rw], gi[:rw], psq[:rw])
                        nc.vector.tensor_add(f1[:rw], f1[:rw], gi[:rw])
                        if use_mask:
                            gcol = mr_off + (c0 + r0) // P
                            mr_col = mrc_sb[:rw, gcol:gcol + 1]
                            t16 = layer_norm_store(
                                f1, rw, c0 + r0, dst, last,
                                l_tile=lr[:, 0, :], mr_col=mr_col,
                                scratch=f2)
                        else:
                            t16 = layer_norm_store(f1, rw, c0 + r0, dst,
                                                   last)
                        out16.append((t16, rw))
                if use_mask:
                    mr_off += max(1, rows // P)
                prev16 = out16 if (rows <= 2 * P and not last) else None
                src, dst = dst, src

    return nc


# ---------------------------------------------------------------------------
# Host side


def _prep_inputs(sequence, input_mask, W_init, b_init, W1, b1, W2, b2,
                 ln_g, ln_b, n_cores):
    """Shard + lay out inputs for the device kernel."""
    N, S, Dd = sequence.shape
    assert Dd == D
    nl = N // n_cores
    R0 = nl * S

    use_mask = not np.all(input_mask == 1.0)
    use_gb = not (np.all(ln_g == 1.0) and np.all(ln_b == 0.0))
    use_b2 = not np.all(b2 == 0.0)
    use_binit = not np.all(b_init == 0.0)

    w1h = np.ascontiguousarray(
        W1.reshape(KT1, P, MT1, P).transpose(2, 1, 0, 3)).astype(np.float16)
    w2h = np.ascontiguousarray(
        W2.reshape(KT2, P, 4 * D).transpose(1, 0, 2)).astype(np.float16)
    winit_h = np.ascontiguousarray(
        W_init.reshape(KT0, P, D).transpose(1, 0, 2)).astype(np.float16)
    b1c = np.ascontiguousarray(b1.reshape(MT1, P).T).astype(np.float32)

    masked = (sequence * input_mask[..., None]).astype(np.float32)

    nlevels = S.bit_length() - 1
    in_maps = []
    IC0 = min(512, R0)
    for c in range(n_cores):
        sl = masked[c * nl:(c + 1) * nl].reshape(R0, D)
        # [p, chunk, kt, r] layout: per-partition contiguous chunk DMAs
        xt0 = np.ascontiguousarray(
            sl.reshape(R0 // IC0, IC0, KT0, P).transpose(3, 0, 2, 1)
        ).astype(np.float16)
        m = {
            "xt0": xt0, "winit": winit_h, "w1": w1h, "w2": w2h, "b1c": b1c,
        }
        if use_gb:
            m["lng"] = ln_g.reshape(1, D).astype(np.float32)
            m["lnb"] = ln_b.reshape(1, D).astype(np.float32)
        if use_b2:
            m["b2r"] = b2.reshape(1, 4 * D).astype(np.float16)
        if use_binit:
            m["bir"] = b_init.reshape(1, D).astype(np.float16)
        if use_mask:
            mc = input_mask[c * nl:(c + 1) * nl].reshape(R0)
            ncols0 = max(1, R0 // P)
            mpad = np.ones(ncols0 * P, np.float32)
            mpad[:R0] = mc
            maskc = np.ascontiguousarray(
                mpad.reshape(ncols0, P).T).astype(np.float32)
            m["maskc"] = maskc
            mr_cols = []
            mcur = mc.copy()
            for lv in range(nlevels):
                half = mcur.shape[0] // 2
                m2 = mcur.reshape(-1, 2)
                mr = m2[:, 1].copy()          # pair (right-child) mask
                mcur = m2[:, 0].copy()        # next-level mask
                ncols = max(1, mr.shape[0] // P)
                pad = np.ones(ncols * P, np.float32)
                pad[:mr.shape[0]] = mr
                mr_cols.append(pad.reshape(ncols, P).T)
            m["mrc"] = np.ascontiguousarray(
                np.concatenate(mr_cols, axis=1)).astype(np.float32)
        in_maps.append(m)

    flags = dict(use_mask=use_mask, use_gb=use_gb, use_b2=use_b2,
                 use_binit=use_binit)
    return in_maps, nl, flags


_GRAPH_CACHE = {}


def _pick_chunk(flags):
    c = 512
    if flags.get("use_gb"):
        c -= 128
    if flags.get("use_b2") or flags.get("use_binit"):
        c -= 128
    return c


def _get_graph(nl, S, **flags):
    chunk = _pick_chunk(flags)
    key = (nl, S, chunk, tuple(sorted(flags.items())))
    if key not in _GRAPH_CACHE:
        _GRAPH_CACHE[key] = build_graph(nl, S, chunk=chunk, **flags)
    return _GRAPH_CACHE[key]


def kernel(sequence, input_mask, W_init, b_init, W1, b1, W2, b2, ln_g, ln_b,
           _trace=False):
    n_cores = 8
    sequence = np.asarray(sequence, dtype=np.float32)
    input_mask = np.asarray(input_mask, dtype=np.float32)
    args = [np.asarray(a, dtype=np.float32)
            for a in (W_init, b_init, W1, b1, W2, b2, ln_g, ln_b)]
    in_maps, nl, flags = _prep_inputs(sequence, input_mask, *args,
                                      n_cores=n_cores)
    N, S, _ = sequence.shape
    nc = _get_graph(nl, S, **flags)
    if not nc.is_finalized():
        nc.finalize()
    res = run_bass_kernel_spmd(nc, in_maps, core_ids=list(range(n_cores)),
                               trace=_trace)
    outs = [res.results[c]["out"] for c in range(n_cores)]
    xfin = np.concatenate(outs, axis=0).reshape(N, 1, D).astype(np.float32)
    global_state = xfin[:, 0, :]
    if _trace:
        kernel._last_exec_time_ns = res.exec_time_ns
        kernel._last_result = res
    return xfin, global_state


# revision 35
# speedup vs baseline: 1.0508x; 1.0508x over previous
"""CRvNN balanced-tree reduction on 8 TRN2 NeuronCores (pure data parallel).

Reference computation (per batch element, S=512 leaves, D=1024, CH=2048):
  x = LN((seq * mask) @ W_init + b_init) * mask
  9 levels of pairwise tree reduction; per level, for each adjacent pair
  (l, r):  cat=[l,r] -> h=relu(cat@W1+b1) -> contents=h@W2+b2 ->
  (f1,f2,i,parent) -> LN(f1*l + f2*r + i*parent) blended by the pair mask.

Distribution: batch 64 -> 8 elements per core, weights replicated; the tree
reduction is independent per batch element, so there are no collectives.

Device kernel (per core, all 9 levels + init fused in one NEFF):
  - fp16 matmuls (PE runs fp16 at 4x the fp32 rate), f32 PSUM accumulate,
    all gating/blending/LayerNorm math in f32 on ACT/DVE.
  - x lives in DRAM between levels as f32 (blend inputs) plus an fp16 mirror
    that feeds DMA-transposed catT tiles for the next level's matmuls.
  - stage 1: hT[CH, rows] = W1^T @ catT, W1 streamed per 128-col slice
    (stationary), catT moving.  relu+b1 fused in the PSUM->SBUF evict.
  - stage 2: contents[rows, 4D] = h @ W2, hT slices stationary, W2
    SBUF-resident (fp16, 16 MiB), evicted through sigmoid into the gates.
"""

import os
import sys

for _p in ("/opt/trn_rl_repo", "/root/.axon_site/_ro/trn_rl_repo"):
    if os.path.isdir(_p) and _p not in sys.path:
        sys.path.append(_p)

import numpy as np

import concourse.bass as bass
import concourse.bacc as bacc_mod
import concourse.mybir as mybir
import concourse.tile as tile
from concourse.bass_utils import run_bass_kernel_spmd
from concourse.masks import make_identity
from concourse.tile_autobufs import add_dep_helper

# ---------------------------------------------------------------------------
# Patch: this walrus build rejects >2 sem waits on one instruction; the Tile
# exit drain carries one wait per active proc.  Split the excess onto SP nops.
from concourse.tile import ScopedClock


def _patched_drain_and_barrier(self, tick_clock, wait_clock):
    nc = self.nc
    drain_inst = nc.sync.drain()
    wait_clock.add_sem_waits(
        drain_inst.ins, ScopedClock({None: tick_clock.global_clock})
    )
    si = drain_inst.ins.sync_info
    waits = si.on_wait if si is not None else None
    extra = []
    while waits is not None and len(waits) > 1:
        extra.append(waits.pop())
    for sw in extra:
        nop = nc.sync.nop(nofuse=True, hint="drain_wait_split")
        nsi = nop.ins.sync_info
        if nsi is None or nsi.on_wait is None:
            nop.ins.sync_info = mybir.SyncInfo(on_wait=[sw], on_update=[])
        else:
            nsi.on_wait.append(sw)

    nc.all_engine_barrier()
    assert self.sems is not None
    popped = nc._tile_sem_poison_stack.pop()
    assert popped is self._sem_poison
    nc.clear_and_free_semaphores(list(self.sems.allocated().values()))
    nc.all_engine_barrier()


tile.TileContext._drain_and_barrier = _patched_drain_and_barrier
# ---------------------------------------------------------------------------

F32 = mybir.dt.float32
F16 = mybir.dt.float16
AF = mybir.ActivationFunctionType
ALU = mybir.AluOpType

D = 1024
CH = 2048
LN_EPS = 1e-5
P = 128
KT1 = (2 * D) // P   # 16 k-tiles for stage 1 / cat
MT1 = CH // P        # 16 m-tiles for stage 1 output (hT partitions)
KT2 = CH // P        # 16 k-tiles for stage 2
KT0 = D // P         # 8 k-tiles for the init matmul


def build_graph(nl, S, chunk=512, use_mask=False, use_gb=False,
                use_b2=False, use_binit=False):
    """Build the per-core Bass graph.  nl = local batch, S = sequence length.

    Flag args compile in the generic paths (pair-mask blending, ln scale/bias,
    matmul biases); they are off for the graded inputs (mask=1, g=1, b=0).
    """
    nlevels = S.bit_length() - 1
    assert S == 1 << nlevels
    R0 = nl * S

    nc = bacc_mod.Bacc()
    IC0 = min(512, R0)
    xt0_ext = nc.declare_dram_parameter("xt0", [P, R0 // IC0, KT0, IC0], F16, isOutput=False)
    winit_ext = nc.declare_dram_parameter("winit", [P, KT0, D], F16, isOutput=False)
    w1_ext = nc.declare_dram_parameter("w1", [MT1, P, KT1, P], F16, isOutput=False)
    w2_ext = nc.declare_dram_parameter("w2", [P, KT2, 4 * D], F16, isOutput=False)
    b1c_ext = nc.declare_dram_parameter("b1c", [P, MT1], F32, isOutput=False)
    if use_gb:
        lng_ext = nc.declare_dram_parameter("lng", [1, D], F32, isOutput=False)
        lnb_ext = nc.declare_dram_parameter("lnb", [1, D], F32, isOutput=False)
    if use_b2:
        b2r_ext = nc.declare_dram_parameter("b2r", [1, 4 * D], F16, isOutput=False)
    if use_binit:
        bir_ext = nc.declare_dram_parameter("bir", [1, D], F16, isOutput=False)
    if use_mask:
        # maskc: per-row mask after init ([P, R0/P] column-tiled)
        # mrc: per-level pair masks, each level padded to whole columns
        maskc_ext = nc.declare_dram_parameter("maskc", [P, max(1, R0 // P)], F32, isOutput=False)
        n_mr_cols = sum(max(1, (nl * (S >> (lv + 1))) // P) for lv in range(nlevels))
        mrc_ext = nc.declare_dram_parameter("mrc", [P, n_mr_cols], F32, isOutput=False)
    out_ext = nc.declare_dram_parameter("out", [nl, D], F32, isOutput=True)

    # DRAM scratch: ping/pong x (f16; feeds both the transposed matmul
    # loads and the l/r blend reads)
    xh = [nc.dram_tensor(f"xh{i}", [R0, D], F16) for i in range(2)]

    with tile.TileContext(nc) as tc:
        import contextlib
        with contextlib.ExitStack() as ctx:
            const = ctx.enter_context(tc.tile_pool(name="const", bufs=1))
            w2p = ctx.enter_context(tc.tile_pool(name="w2p", bufs=1))
            w1p = ctx.enter_context(tc.tile_pool(name="w1p", bufs=2))
            catp = ctx.enter_context(tc.tile_pool(name="catp", bufs=1))
            htp = ctx.enter_context(tc.tile_pool(name="htp", bufs=1))
            lrp = ctx.enter_context(tc.tile_pool(name="lrp", bufs=1))
            gp = ctx.enter_context(tc.tile_pool(name="gp", bufs=1))
            op_bufs = 1 if (use_gb or use_b2 or use_binit) else 2
            op = ctx.enter_context(tc.tile_pool(name="op", bufs=op_bufs))
            sp = ctx.enter_context(tc.tile_pool(name="sp", bufs=4))
            ps1 = ctx.enter_context(tc.tile_pool(name="ps1", bufs=2, space="PSUM"))
            ps2 = ctx.enter_context(tc.tile_pool(name="ps2", bufs=3, space="PSUM"))

            # --- resident constants -------------------------------------
            # (the 16 MiB w2 load is EMITTED after the init loop so it does
            # not head-of-line-block the init's input DMAs; it is only
            # needed from level 1 stage 2 onward)
            w2_sb = w2p.tile([P, KT2, 4 * D], F16)
            # winit borrows the (init-time idle) hT slot: same 16 KiB/part
            winit_sb = htp.tile([P, KT0, D], F16, tag="hT")
            nc.sync.dma_start(winit_sb[:], winit_ext[:])
            b1c_sb = const.tile([P, MT1], F32)
            eps_sb = const.tile([P, 1], F32)
            nc.vector.memset(eps_sb[:], LN_EPS)
            ident_sb = const.tile([P, P], F16)
            make_identity(nc, ident_sb)
            if use_gb:
                g_sb = const.tile([P, D], F32)
                b_sb = const.tile([P, D], F32)
                nc.sync.dma_start(g_sb[:], lng_ext[:].to_broadcast((P, D)))
                nc.sync.dma_start(b_sb[:], lnb_ext[:].to_broadcast((P, D)))
            if use_b2:
                b2_sb = const.tile([1, 4 * D], F16)
                nc.sync.dma_start(b2_sb[:], b2r_ext[:])
            if use_binit:
                bir_sb = const.tile([1, D], F16)
                nc.sync.dma_start(bir_sb[:], bir_ext[:])
            if use_b2 or use_binit:
                ones_sb = const.tile([1, P], F16)
                nc.vector.memset(ones_sb[:], 1.0)
            if use_mask:
                maskc_sb = const.tile([P, max(1, R0 // P)], F32)
                nc.sync.dma_start(maskc_sb[:], maskc_ext[:])
                mrc_sb = const.tile([P, n_mr_cols], F32)
                nc.sync.dma_start(mrc_sb[:], mrc_ext[:])

            def layer_norm_store(x_in, rw, rt0, dst_idx, last, mask_col=None,
                                 l_tile=None, mr_col=None, scratch=None):
                """LN(x_in[:rw]) (+g/b), optional mask blend with l_tile,
                then store f32 + f16 mirrors (or `out` on the last level).
                x_in may be SBUF or PSUM; scratch: SBUF f32 tile for temps.
                Returns the f16 output tile (None on the last level)."""
                stats = sp.tile([P, 2, 6], F32)
                mv = sp.tile([P, 2], F32)
                nc.vector.bn_stats(out=stats[:rw, 0, :], in_=x_in[:rw, 0:512])
                nc.vector.bn_stats(out=stats[:rw, 1, :], in_=x_in[:rw, 512:1024])
                nc.vector.bn_aggr(out=mv[:rw], in_=stats[:rw])
                rs = sp.tile([P, 1], F32)
                nc.scalar.activation(rs[:rw], mv[:rw, 1:2], AF.Sqrt, bias=eps_sb[:rw])
                nc.vector.reciprocal(rs[:rw], rs[:rw])
                direct16 = not (last or use_gb or mask_col is not None
                                or mr_col is not None)
                if direct16:
                    xo16 = op.tile([P, D], F16)
                    nc.vector.tensor_scalar(
                        out=xo16[:rw], in0=x_in[:rw],
                        scalar1=mv[:rw, 0:1], scalar2=rs[:rw],
                        op0=ALU.subtract, op1=ALU.mult,
                    )
                    nc.sync.dma_start(xh[dst_idx][rt0:rt0 + rw, :], xo16[:rw])
                    return xo16
                xo = op.tile([P, D], F32)
                nc.vector.tensor_scalar(
                    out=xo[:rw], in0=x_in[:rw],
                    scalar1=mv[:rw, 0:1], scalar2=rs[:rw],
                    op0=ALU.subtract, op1=ALU.mult,
                )
                if use_gb:
                    nc.vector.tensor_mul(xo[:rw], xo[:rw], g_sb[:rw])
                    nc.vector.tensor_add(xo[:rw], xo[:rw], b_sb[:rw])
                if mask_col is not None:
                    # init re-mask: x *= mask
                    nc.vector.tensor_scalar_mul(xo[:rw], xo[:rw], mask_col)
                if mr_col is not None:
                    # x = l + mr*(x - l)
                    t = scratch
                    nc.vector.tensor_sub(t[:rw], xo[:rw], l_tile[:rw])
                    nc.vector.tensor_scalar_mul(t[:rw], t[:rw], mr_col)
                    nc.vector.tensor_add(xo[:rw], l_tile[:rw], t[:rw])
                if last:
                    nc.sync.dma_start(out_ext[rt0:rt0 + rw, :], xo[:rw])
                    return None
                xo16 = op.tile([P, D], F16)
                nc.vector.tensor_copy(xo16[:rw], xo[:rw])
                nc.sync.dma_start(xh[dst_idx][rt0:rt0 + rw, :], xo16[:rw])
                return xo16

            # --- init: y = LN((seq*mask) @ W_init + b_init) * mask ------
            # one persistent tile, ping-pong between column halves so the
            # next chunk's DMA overlaps this chunk's matmuls
            IC = IC0
            xt0_sb = catp.tile([P, KT0, 2, IC], F16, tag="catT")
            n_init_chunks = (R0 + IC - 1) // IC
            xt0_dmas = []
            for ci, c0 in enumerate(range(0, R0, IC)):
                cw = min(IC, R0 - c0)
                half = ci % 2
                xt0_c = xt0_sb[:, :, half, :]
                xt0_dmas.append(nc.sync.dma_start(
                    xt0_c[:, :, :cw], xt0_ext[:, ci, :, :cw]
                ))

                for rt in range((cw + P - 1) // P):
                    rw = min(P, cw - rt * P)
                    r0 = rt * P
                    psy = ps2.tile([P, D], F32, tag="psq")
                    for k in range(KT0):
                        st = (k == 0)
                        sp_ = (k == KT0 - 1) and not use_binit
                        nc.tensor.matmul(
                            psy[:rw, 0:512],
                            xt0_c[:, k, r0:r0 + rw],
                            winit_sb[:, k, 0:512], start=st, stop=sp_)
                        nc.tensor.matmul(
                            psy[:rw, 512:1024],
                            xt0_c[:, k, r0:r0 + rw],
                            winit_sb[:, k, 512:1024], start=st, stop=sp_)
                    if use_binit:
                        nc.tensor.matmul(psy[:rw, 0:512], ones_sb[:1, :rw],
                                         bir_sb[:1, 0:512], start=False, stop=True)
                        nc.tensor.matmul(psy[:rw, 512:1024], ones_sb[:1, :rw],
                                         bir_sb[:1, 512:1024], start=False, stop=True)
                    # quick ACT evict: frees PSUM sooner, LN runs on SBUF
                    yw = gp.tile([P, D], F32, tag="gate0")
                    nc.scalar.activation(yw[:rw], psy[:rw], AF.Copy)
                    gcol = (c0 + r0) // P
                    layer_norm_store(
                        yw, rw, c0 + r0, 0, last=False,
                        mask_col=maskc_sb[:rw, gcol:gcol + 1] if use_mask else None,
                    )

            # deferred resident loads (needed from level 1 onward); the
            # 16 MiB w2 load rides the SWDGE queues so it cannot starve
            # the init input stream on the HWDGE queues
            # pace the 16 MiB w2 residency load across the init phase in
            # four quarters (SWDGE queues, dep-chained to the init stream)
            # so it never monopolizes HBM
            nq = KT2 // 4
            for qi in range(4):
                w2dma = nc.gpsimd.dma_start(
                    w2_sb[:, qi * nq:(qi + 1) * nq, :],
                    w2_ext[:, qi * nq:(qi + 1) * nq, :])
                gate = min((qi + 1) * (len(xt0_dmas) // 4), len(xt0_dmas)) - 1
                add_dep_helper(w2dma.ins, xt0_dmas[max(gate, 0)].ins,
                               reason="paced w2 residency load")
            nc.sync.dma_start(b1c_sb[:], b1c_ext[:])

            # --- tree levels -------------------------------------------
            src, dst = 0, 1
            mr_off = 0
            prev16 = None   # f16 output tiles of the previous level (tail)
            for lv in range(nlevels):
                rows = nl * (S >> (lv + 1))   # parent count this level
                rows_x = 2 * rows             # child count
                last = (lv == nlevels - 1)
                # tail levels: children live in <=2 SBUF tiles; transpose
                # on the PE instead of round-tripping through DRAM
                onchip = (rows_x <= 2 * P and prev16 is not None
                          and len(prev16) * P >= rows_x)
                cat_view = xh[src][:].rearrange("(r two) d -> r (two d)", two=2)
                out16 = []
                xT = None
                if onchip:
                    xT = catp.tile([P, KT0, 2 * P], F16, tag="catT")
                    for t, (t16, cwt) in enumerate(prev16):
                        for kk in range(KT0):
                            tp = ps1.tile([P, P], F16, tag="psh")
                            nc.tensor.transpose(
                                tp[:P, :cwt], t16[:cwt, kk * P:(kk + 1) * P],
                                ident_sb[:cwt, :cwt])
                            nc.vector.tensor_copy(
                                xT[:, kk, t * P:t * P + cwt], tp[:, :cwt])
                for c0 in range(0, rows, chunk):
                    cw = min(chunk, rows - c0)
                    if not onchip:
                        # catT tiles via DMA transpose from the f16 mirror;
                        # split by row halves so the first half's transposes
                        # start before the previous level's last rows land
                        catT = catp.tile([P, KT1, chunk], F16, tag="catT")
                        cw2 = cw // 2 if cw >= 32 else cw
                        for k in range(KT1):
                            for h0 in range(0, cw, cw2):
                                hw = min(cw2, cw - h0)
                                nc.sync.dma_start_transpose(
                                    catT[:, k, h0:h0 + hw],
                                    cat_view[c0 + h0:c0 + h0 + hw,
                                             k * P:(k + 1) * P],
                                )
                    # stage 1: hT = relu(W1^T @ catT + b1)
                    hT = htp.tile([P, MT1, chunk], F16, tag="hT")
                    for m in range(MT1):
                        w1m = w1p.tile([P, KT1, P], F16)
                        nc.sync.dma_start(w1m[:], w1_ext[m, :, :, :])
                        psh = ps1.tile([P, chunk], F32, tag="psh")
                        for k in range(KT1):
                            if onchip:
                                ph = 0 if k < KT0 else 1
                                rhs = xT[:, k % KT0, ph:ph + 2 * cw - 1:2]
                            else:
                                rhs = catT[:, k, :cw]
                            nc.tensor.matmul(
                                psh[:, :cw], w1m[:, k, :], rhs,
                                start=(k == 0), stop=(k == KT1 - 1))
                        nc.scalar.activation(
                            hT[:, m, :cw], psh[:, :cw], AF.Relu,
                            bias=b1c_sb[:, m:m + 1])
                    # stage 2 + post, per 128-row tile
                    for rt in range((cw + P - 1) // P):
                        rw = min(P, cw - rt * P)
                        r0 = rt * P
                        gates = []   # f1, f2, i in SBUF; parent stays in PSUM
                        psq = None
                        for q in range(4):
                            psq = ps2.tile([P, D], F32, tag="psq")
                            for k in range(KT2):
                                st = (k == 0)
                                sp_ = (k == KT2 - 1) and not use_b2
                                nc.tensor.matmul(
                                    psq[:rw, 0:512],
                                    hT[:, k, r0:r0 + rw],
                                    w2_sb[:, k, q * D:q * D + 512],
                                    start=st, stop=sp_)
                                nc.tensor.matmul(
                                    psq[:rw, 512:1024],
                                    hT[:, k, r0:r0 + rw],
                                    w2_sb[:, k, q * D + 512:(q + 1) * D],
                                    start=st, stop=sp_)
                            if use_b2:
                                nc.tensor.matmul(
                                    psq[:rw, 0:512], ones_sb[:1, :rw],
                                    b2_sb[:1, q * D:q * D + 512],
                                    start=False, stop=True)
                                nc.tensor.matmul(
                                    psq[:rw, 512:1024], ones_sb[:1, :rw],
                                    b2_sb[:1, q * D + 512:(q + 1) * D],
                                    start=False, stop=True)
                            if q < 3:
                                gt = gp.tile([P, D], F32, tag=f"gate{q}")
                                nc.scalar.activation(gt[:rw], psq[:rw], AF.Sigmoid)
                                gates.append(gt)
                        # parent candidate = psq (q==3, raw)
                        lr = lrp.tile([P, 2, D], F16)
                        nc.sync.dma_start(
                            lr[:rw, 0, :],
                            xh[src][2 * (c0 + r0):2 * (c0 + r0 + rw):2, :])
                        nc.sync.dma_start(
                            lr[:rw, 1, :],
                            xh[src][2 * (c0 + r0) + 1:2 * (c0 + r0 + rw):2, :])
                        f1, f2, gi = gates
                        nc.vector.tensor_mul(f1[:rw], f1[:rw], lr[:rw, 0, :])
                        nc.vector.tensor_mul(f2[:rw], f2[:rw], lr[:rw, 1, :])
                        nc.vector.tensor_add(f1[:rw], f1[:rw], f2[:rw])
                        nc.vector.tensor_mul(gi[:rw], gi[:rw], psq[:rw])
                        nc.vector.tensor_add(f1[:rw], f1[:rw], gi[:rw])
                        if use_mask:
                            gcol = mr_off + (c0 + r0) // P
                            mr_col = mrc_sb[:rw, gcol:gcol + 1]
                            t16 = layer_norm_store(
                                f1, rw, c0 + r0, dst, last,
                                l_tile=lr[:, 0, :], mr_col=mr_col,
                                scratch=f2)
                        else:
                            t16 = layer_norm_store(f1, rw, c0 + r0, dst,
                                                   last)
                        out16.append((t16, rw))
                if use_mask:
                    mr_off += max(1, rows // P)
                prev16 = out16 if (rows <= 2 * P and not last) else None
                src, dst = dst, src

    return nc


# ---------------------------------------------------------------------------
# Host side


def _prep_inputs(sequence, input_mask, W_init, b_init, W1, b1, W2, b2,
                 ln_g, ln_b, n_cores):
    """Shard + lay out inputs for the device kernel."""
    N, S, Dd = sequence.shape
    assert Dd == D
    nl = N // n_cores
    R0 = nl * S

    use_mask = not np.all(input_mask == 1.0)
    use_gb = not (np.all(ln_g == 1.0) and np.all(ln_b == 0.0))
    use_b2 = not np.all(b2 == 0.0)
    use_binit = not np.all(b_init == 0.0)

    w1h = np.ascontiguousarray(
        W1.reshape(KT1, P, MT1, P).transpose(2, 1, 0, 3)).astype(np.float16)
    w2h = np.ascontiguousarray(
        W2.reshape(KT2, P, 4 * D).transpose(1, 0, 2)).astype(np.float16)
    winit_h = np.ascontiguousarray(
        W_init.reshape(KT0, P, D).transpose(1, 0, 2)).astype(np.float16)
    b1c = np.ascontiguousarray(b1.reshape(MT1, P).T).astype(np.float32)

    masked = (sequence * input_mask[..., None]).astype(np.float32)

    nlevels = S.bit_length() - 1
    in_maps = []
    IC0 = min(512, R0)
    for c in range(n_cores):
        sl = masked[c * nl:(c + 1) * nl].reshape(R0, D)
        # [p, chunk, kt, r] layout: per-partition contiguous chunk DMAs
        xt0 = np.ascontiguousarray(
            sl.reshape(R0 // IC0, IC0, KT0, P).transpose(3, 0, 2, 1)
        ).astype(np.float16)
        m = {
            "xt0": xt0, "winit": winit_h, "w1": w1h, "w2": w2h, "b1c": b1c,
        }
        if use_gb:
            m["lng"] = ln_g.reshape(1, D).astype(np.float32)
            m["lnb"] = ln_b.reshape(1, D).astype(np.float32)
        if use_b2:
            m["b2r"] = b2.reshape(1, 4 * D).astype(np.float16)
        if use_binit:
            m["bir"] = b_init.reshape(1, D).astype(np.float16)
        if use_mask:
            mc = input_mask[c * nl:(c + 1) * nl].reshape(R0)
            ncols0 = max(1, R0 // P)
            mpad = np.ones(ncols0 * P, np.float32)
            mpad[:R0] = mc
            maskc = np.ascontiguousarray(
                mpad.reshape(ncols0, P).T).astype(np.float32)
            m["maskc"] = maskc
            mr_cols = []
            mcur = mc.copy()
            for lv in range(nlevels):
                half = mcur.shape[0] // 2
                m2 = mcur.reshape(-1, 2)
                mr = m2[:, 1].copy()          # pair (right-child) mask
                mcur = m2[:, 0].copy()        # next-level mask
                ncols = max(1, mr.shape[0] // P)
                pad = np.ones(ncols * P, np.float32)
                pad[:mr.shape[0]] = mr
                mr_cols.append(pad.reshape(ncols, P).T)
            m["mrc"] = np.ascontiguousarray(
                np.concatenate(mr_cols, axis=1)).astype(np.float32)
        in_maps.append(m)

    flags = dict(use_mask=use_mask, use_gb=use_gb, use_b2=use_b2,
                 use_binit=use_binit)
    return in_maps, nl, flags


_GRAPH_CACHE = {}


def _pick_chunk(flags):
    c = 512
    if flags.get("use_gb"):
        c -= 128
    if flags.get("use_b2") or flags.get("use_binit"):
        c -= 128
    return c


def _get_graph(nl, S, **flags):
    chunk = _pick_chunk(flags)
    key = (nl, S, chunk, tuple(sorted(flags.items())))
    if key not in _GRAPH_CACHE:
        _GRAPH_CACHE[key] = build_graph(nl, S, chunk=chunk, **flags)
    return _GRAPH_CACHE[key]


def kernel(sequence, input_mask, W_init, b_init, W1, b1, W2, b2, ln_g, ln_b,
           _trace=False):
    n_cores = 8
    sequence = np.asarray(sequence, dtype=np.float32)
    input_mask = np.asarray(input_mask, dtype=np.float32)
    args = [np.asarray(a, dtype=np.float32)
            for a in (W_init, b_init, W1, b1, W2, b2, ln_g, ln_b)]
    in_maps, nl, flags = _prep_inputs(sequence, input_mask, *args,
                                      n_cores=n_cores)
    N, S, _ = sequence.shape
    nc = _get_graph(nl, S, **flags)
    if not nc.is_finalized():
        nc.finalize()
    res = run_bass_kernel_spmd(nc, in_maps, core_ids=list(range(n_cores)),
                               trace=_trace)
    outs = [res.results[c]["out"] for c in range(n_cores)]
    xfin = np.concatenate(outs, axis=0).reshape(N, 1, D).astype(np.float32)
    global_state = xfin[:, 0, :]
    if _trace:
        kernel._last_exec_time_ns = res.exec_time_ns
        kernel._last_result = res
    return xfin, global_state
